# revision 11
# baseline (speedup 1.0000x reference)
"""3x3 valid conv (cross-correlation) + bias on a 4096x4096 f32 image.

Sharding: rows across 8 NeuronCores (512 output rows each); the (kH-1)-row
halo is provided host-side by overlapping the per-core input slabs, so no
device collective is needed. The image is zero-padded to 4098x4098 so all
cores run one uniform SPMD program; the pad region is trimmed on gather.

Per-core compute (tensor engine): for each column shift b in {0,1,2}, a
banded [K=M+2, M] matrix B_b with B_b[m+a, m] = w[a, b] folds all three
row taps into the K-contraction:

    (B_b.T @ X_rows)[m, n] = sum_a w[a, b] * X[m+a, n]

Accumulating the three column-shifted views of the moving tensor into one
PSUM bank yields the full 3x3 conv in 3 matmuls per [126, 512] tile.
Inputs are fed as fp32r: 1 PE cycle/row at N=512 (vs 4 for fp32), at the
cost of ~12-bit operand rounding (~2e-4 scale-relative output error,
resid_var ~1e-8). Bias is fused into the PSUM->SBUF eviction, which
alternates between the scalar and vector engines.

DMA layout (the kernel is memory-bound, ~47 us/core of traffic at
360 GB/s): input loads ride the SP HWDGE ring, output stores the ACT
HWDGE ring, so store sem-waits never head-of-line-block loads; each row
group is split into two independent single-writer column-half tiles with
6-deep pools so loads run ~3 groups ahead. Cost-model timeline: ~52 us
per core.
"""

import sys

if "/opt/trn_rl_repo" not in sys.path:
    sys.path.insert(0, "/opt/trn_rl_repo")

import numpy as np

import concourse.bacc as bacc
import concourse.mybir as mybir
from concourse import tile
from concourse.bass_utils import run_bass_kernel_spmd

N_CORES = 8
H, W = 4096, 4096
KH, KW = 3, 3
HALO = 2  # KH - 1
OUT_ROWS = 512  # output rows per core (padded output H = 4096)
IN_ROWS = OUT_ROWS + HALO  # 514
W_PAD = W + HALO  # 4098: lets every core compute a full 4096-wide output
M_TILE = 126  # output rows per matmul (K = M + 2 <= 128)
N_TILE = 512  # matmul free dim = one PSUM bank of f32

_CACHE = {}


def _build_program():
    f32 = mybir.dt.float32
    f32r = mybir.dt.float32r

    nc = bacc.Bacc(
        "TRN2", target_bir_lowering=False, debug=False, num_devices=N_CORES
    )
    x = nc.declare_dram_parameter("x", [IN_ROWS, W_PAD], f32, isOutput=False)
    wb = nc.declare_dram_parameter("wb", [128, KW, M_TILE], f32, isOutput=False)
    bias = nc.declare_dram_parameter("bias", [128, 1], f32, isOutput=False)
    out = nc.declare_dram_parameter("out", [OUT_ROWS, W], f32, isOutput=True)

    # row groups: 4 x 126 + 1 x 8 = 512
    groups = []
    m0 = 0
    while m0 < OUT_ROWS:
        m = min(M_TILE, OUT_ROWS - m0)
        groups.append((m0, m))
        m0 += m

    n_cols = W // N_TILE  # 8 column tiles
    half = n_cols // 2
    xhalf = half * N_TILE + HALO  # 2050: input cols per half (2-col overlap)

    with tile.TileContext(nc) as tc:
        with (
            tc.tile_pool(name="const", bufs=1) as cpool,
            tc.tile_pool(name="xin", bufs=6) as xpool,
            tc.tile_pool(name="psum", bufs=8, space="PSUM") as ppool,
            tc.tile_pool(name="oput", bufs=6) as opool,
        ):
            wt = cpool.tile([128, KW, M_TILE], f32r)
            nc.sync.dma_start(wt[:], wb[:].bitcast(f32r))
            bt = cpool.tile([128, 1], f32)
            nc.sync.dma_start(bt[:], bias[:])

            for m0, m in groups:
                k = m + HALO
                for h in range(2):
                    # independent single-writer tiles per column half
                    xc0 = h * half * N_TILE  # 0 or 2048
                    xt = xpool.tile([128, xhalf], f32r, tag="xin")
                    nc.sync.dma_start(
                        xt[:k, :], x[m0 : m0 + k, xc0 : xc0 + xhalf].bitcast(f32r)
                    )
                    ot = opool.tile([128, half * N_TILE], f32, tag="oput")
                    for jj in range(half):
                        c0 = jj * N_TILE
                        pt = ppool.tile([128, N_TILE], f32)
                        for b in range(KW):
                            nc.tensor.matmul(
                                pt[:m, :],
                                wt[:k, b, :m],
                                xt[:k, c0 + b : c0 + b + N_TILE],
                                start=(b == 0),
                                stop=(b == KW - 1),
                            )
                        # evict psum+bias to SBUF; alternate ACT/DVE
                        if jj % 2 == 0:
                            nc.scalar.activation(
                                ot[:m, c0 : c0 + N_TILE],
                                pt[:m, :],
                                mybir.ActivationFunctionType.Identity,
                                bias=bt[:m],
                                scale=1.0,
                            )
                        else:
                            nc.vector.tensor_scalar_add(
                                ot[:m, c0 : c0 + N_TILE], pt[:m, :], bt[:m]
                            )
                    # stores go on the ACT HWDGE queue so their sem waits
                    # don't head-of-line-block input loads on the SP queue
                    nc.scalar.dma_start(
                        out[m0 : m0 + m, xc0 : xc0 + half * N_TILE], ot[:m, :]
                    )

    nc.compile()
    return nc


def kernel(X: np.ndarray, weight: np.ndarray, bias: np.ndarray) -> np.ndarray:
    X = np.ascontiguousarray(X, dtype=np.float32)
    weight = np.asarray(weight, dtype=np.float32)
    bias = np.asarray(bias, dtype=np.float32)

    if "nc" not in _CACHE:
        _CACHE["nc"] = _build_program()
    nc = _CACHE["nc"]

    # host-side prep (tiny): padded image, banded weights, broadcast bias
    x_pad = np.zeros((H + HALO, W_PAD), dtype=np.float32)
    x_pad[:H, :W] = X

    wb = np.zeros((128, KW, M_TILE), dtype=np.float32)
    m_idx = np.arange(M_TILE)
    for b in range(KW):
        for a in range(KH):
            wb[m_idx + a, b, m_idx] = weight[a, b]

    bias_bc = np.full((128, 1), bias[0], dtype=np.float32)

    in_maps = [
        {
            "x": x_pad[c * OUT_ROWS : c * OUT_ROWS + IN_ROWS],
            "wb": wb,
            "bias": bias_bc,
        }
        for c in range(N_CORES)
    ]

    try:
        res = run_bass_kernel_spmd(nc, in_maps, core_ids=list(range(N_CORES)))
    except ModuleNotFoundError:
        # BASS_TRACE=1 requires the axon NTFF hook (antenv.axon_hooks),
        # which some containers lack -- fall back to an untraced run
        import os

        os.environ["BASS_NEVER_TRACE"] = "1"
        res = run_bass_kernel_spmd(nc, in_maps, core_ids=list(range(N_CORES)))
    _CACHE["last_results"] = res  # exec_time_ns when BASS_TRACE=1
    full = np.concatenate([r["out"] for r in res.results], axis=0)
    return np.ascontiguousarray(full[: H - KH + 1, : W - KW + 1])


# revision 12
# speedup vs baseline: 1.2973x; 1.2973x over previous
"""3x3 valid conv (cross-correlation) + bias on a 4096x4096 f32 image.

Sharding: rows across 8 NeuronCores (512 output rows each); the (kH-1)-row
halo is provided host-side by overlapping the per-core input slabs, so no
device collective is needed. The image is zero-padded to 4098x4098 so all
cores run one uniform SPMD program; the pad region is trimmed on gather.

Per-core compute (tensor engine): for each column shift b in {0,1,2}, a
banded [K=M+2, M] matrix B_b with B_b[m+a, m] = w[a, b] folds all three
row taps into the K-contraction:

    (B_b.T @ X_rows)[m, n] = sum_a w[a, b] * X[m+a, n]

Accumulating the three column-shifted views of the moving tensor into one
PSUM bank yields the full 3x3 conv in 3 matmuls per [126, 512] tile.
Inputs are fed as fp32r: 1 PE cycle/row at N=512 (vs 4 for fp32), at the
cost of ~12-bit operand rounding (~2e-4 scale-relative output error,
resid_var ~1e-8). Bias is fused into the PSUM->SBUF eviction, which
alternates between the scalar and vector engines.

DMA layout (the kernel is memory-bound, ~47 us/core of traffic at
360 GB/s): input loads ride the SP HWDGE ring, output stores the ACT
HWDGE ring, so store sem-waits never head-of-line-block loads; each row
group is split into two independent single-writer column-half tiles with
6-deep pools so loads run ~3 groups ahead. Cost-model timeline: ~52 us
per core.
"""

import sys

if "/opt/trn_rl_repo" not in sys.path:
    sys.path.insert(0, "/opt/trn_rl_repo")

import numpy as np

import concourse.bacc as bacc
import concourse.mybir as mybir
from concourse import tile
from concourse.bass_utils import run_bass_kernel_spmd

N_CORES = 8
H, W = 4096, 4096
KH, KW = 3, 3
HALO = 2  # KH - 1
OUT_ROWS = 512  # output rows per core (padded output H = 4096)
IN_ROWS = OUT_ROWS + HALO  # 514
W_PAD = W + HALO  # 4098: lets every core compute a full 4096-wide output
M_TILE = 126  # output rows per matmul (K = M + 2 <= 128)
N_TILE = 512  # matmul free dim = one PSUM bank of f32

_CACHE = {}


def _build_program():
    f32 = mybir.dt.float32
    f32r = mybir.dt.float32r

    nc = bacc.Bacc(
        "TRN2", target_bir_lowering=False, debug=False, num_devices=N_CORES
    )
    f16 = mybir.dt.float16
    x = nc.declare_dram_parameter("x", [IN_ROWS, W_PAD], f16, isOutput=False)
    wb = nc.declare_dram_parameter("wb", [128, KW, M_TILE], f16, isOutput=False)
    bias = nc.declare_dram_parameter("bias", [128, 1], f32, isOutput=False)
    out = nc.declare_dram_parameter("out", [OUT_ROWS, W], f32, isOutput=True)

    # row groups: 4 x 126 + 1 x 8 = 512
    groups = []
    m0 = 0
    while m0 < OUT_ROWS:
        m = min(M_TILE, OUT_ROWS - m0)
        groups.append((m0, m))
        m0 += m

    n_cols = W // N_TILE  # 8 column tiles
    half = n_cols // 2
    xhalf = half * N_TILE + HALO  # 2050: input cols per half (2-col overlap)

    with tile.TileContext(nc) as tc:
        with (
            tc.tile_pool(name="const", bufs=1) as cpool,
            tc.tile_pool(name="xin", bufs=6) as xpool,
            tc.tile_pool(name="psum", bufs=8, space="PSUM") as ppool,
            tc.tile_pool(name="oput", bufs=6) as opool,
        ):
            wt = cpool.tile([128, KW, M_TILE], f16)
            nc.sync.dma_start(wt[:], wb[:])
            bt = cpool.tile([128, 1], f32)
            nc.sync.dma_start(bt[:], bias[:])

            for m0, m in groups:
                k = m + HALO
                for h in range(2):
                    # independent single-writer tiles per column half
                    xc0 = h * half * N_TILE  # 0 or 2048
                    xt = xpool.tile([128, xhalf], f16, tag="xin")
                    nc.sync.dma_start(
                        xt[:k, :], x[m0 : m0 + k, xc0 : xc0 + xhalf]
                    )
                    ot = opool.tile([128, half * N_TILE], f32, tag="oput")
                    for jj in range(half):
                        c0 = jj * N_TILE
                        pt = ppool.tile([128, N_TILE], f32)
                        for b in range(KW):
                            nc.tensor.matmul(
                                pt[:m, :],
                                wt[:k, b, :m],
                                xt[:k, c0 + b : c0 + b + N_TILE],
                                start=(b == 0),
                                stop=(b == KW - 1),
                            )
                        # evict psum+bias to SBUF; alternate ACT/DVE
                        if jj % 2 == 0:
                            nc.scalar.activation(
                                ot[:m, c0 : c0 + N_TILE],
                                pt[:m, :],
                                mybir.ActivationFunctionType.Identity,
                                bias=bt[:m],
                                scale=1.0,
                            )
                        else:
                            nc.vector.tensor_scalar_add(
                                ot[:m, c0 : c0 + N_TILE], pt[:m, :], bt[:m]
                            )
                    # stores go on the ACT HWDGE queue so their sem waits
                    # don't head-of-line-block input loads on the SP queue
                    nc.scalar.dma_start(
                        out[m0 : m0 + m, xc0 : xc0 + half * N_TILE], ot[:m, :]
                    )

    nc.compile()
    return nc


def kernel(X: np.ndarray, weight: np.ndarray, bias: np.ndarray) -> np.ndarray:
    X = np.ascontiguousarray(X, dtype=np.float32)
    weight = np.asarray(weight, dtype=np.float32)
    bias = np.asarray(bias, dtype=np.float32)

    if "nc" not in _CACHE:
        _CACHE["nc"] = _build_program()
    nc = _CACHE["nc"]

    # host-side prep (tiny): padded image, banded weights, broadcast bias
    x_pad = np.zeros((H + HALO, W_PAD), dtype=np.float16)
    x_pad[:H, :W] = X.astype(np.float16)

    wb = np.zeros((128, KW, M_TILE), dtype=np.float16)
    m_idx = np.arange(M_TILE)
    for b in range(KW):
        for a in range(KH):
            wb[m_idx + a, b, m_idx] = weight[a, b].astype(np.float16)

    bias_bc = np.full((128, 1), bias[0], dtype=np.float32)

    in_maps = [
        {
            "x": x_pad[c * OUT_ROWS : c * OUT_ROWS + IN_ROWS],
            "wb": wb,
            "bias": bias_bc,
        }
        for c in range(N_CORES)
    ]

    try:
        res = run_bass_kernel_spmd(nc, in_maps, core_ids=list(range(N_CORES)))
    except ModuleNotFoundError:
        # BASS_TRACE=1 requires the axon NTFF hook (antenv.axon_hooks),
        # which some containers lack -- fall back to an untraced run
        import os

        os.environ["BASS_NEVER_TRACE"] = "1"
        res = run_bass_kernel_spmd(nc, in_maps, core_ids=list(range(N_CORES)))
    _CACHE["last_results"] = res  # exec_time_ns when BASS_TRACE=1
    full = np.concatenate([r["out"] for r in res.results], axis=0)
    return np.ascontiguousarray(full[: H - KH + 1, : W - KW + 1])


# revision 23
# speedup vs baseline: 1.6015x; 1.2345x over previous
"""3x3 valid conv (cross-correlation) + bias on a 4096x4096 f32 image.

Sharding: a 4x2 grid over 8 NeuronCores -- 4 row-bands x 2 column-bands of
1024x2048 output each; the (kH-1) halo is provided host-side by
overlapping the per-core input slabs, so no device collective is needed.
The image is zero-padded to 4098x4098 so all cores run one uniform SPMD
program; pad regions are trimmed on gather. 4x2 beats 8x1 on the tensor
engine: the ragged <126-row tail group is paid once per 1024 rows instead
of once per 512 (108 matmuls/core vs 120).

Per-core compute (tensor engine): for each column shift b in {0,1,2}, a
banded [K=M+2, M] matrix B_b with B_b[m+a, m] = w[a, b] folds all three
row taps into the K-contraction:

    (B_b.T @ X_rows)[m, n] = sum_a w[a, b] * X[m+a, n]

Accumulating the three column-shifted views of the moving tensor into one
PSUM bank yields the full 3x3 conv in 3 matmuls per [126, 512] tile.
I/O is fp16 with f32 PSUM accumulation: the PE's fast fp32 path (fp32r)
already rounds operands to ~12 mantissa bits, so fp16 inputs cost almost
nothing extra, and the fp16 store (upcast to f32 on host) trades ~2x
absmax error (5.8e-4 scale-relative, resid_var 7e-8 -- still ~1e3 inside
the 1e-4 resid_var gate) for half the store traffic. Bias is fused into
the PSUM->SBUF eviction, which alternates between scalar and vector
engines.

DMA layout (4.3 MB in + 4.2 MB out per core at 360 GB/s): input loads
ride the SP HWDGE ring, output stores the ACT HWDGE ring, so store
sem-waits never head-of-line-block loads; 4-deep pools let loads run
groups ahead. Cost-model timeline: ~30 us per core.
"""

import sys

if "/opt/trn_rl_repo" not in sys.path:
    sys.path.insert(0, "/opt/trn_rl_repo")

import numpy as np

import concourse.bacc as bacc
import concourse.mybir as mybir
from concourse import tile
from concourse.bass_utils import run_bass_kernel_spmd

N_CORES = 8
GRID_R, GRID_C = 4, 2  # 4 row-bands x 2 col-bands
H, W = 4096, 4096
KH, KW = 3, 3
HALO = 2  # KH - 1
OUT_ROWS = H // GRID_R  # 1024 output rows per core (padded)
OUT_COLS = W // GRID_C  # 2048 output cols per core (padded)
IN_ROWS = OUT_ROWS + HALO  # 1026
IN_COLS = OUT_COLS + HALO  # 2050
M_TILE = 126  # output rows per matmul (K = M + 2 <= 128)
N_TILE = 512  # matmul free dim = one PSUM bank of f32

_CACHE = {}


def _build_program():
    f32 = mybir.dt.float32
    f16 = mybir.dt.float16

    nc = bacc.Bacc(
        "TRN2", target_bir_lowering=False, debug=False, num_devices=N_CORES
    )
    x = nc.declare_dram_parameter("x", [IN_ROWS, IN_COLS], f16, isOutput=False)
    wb = nc.declare_dram_parameter("wb", [128, KW, M_TILE], f16, isOutput=False)
    bias = nc.declare_dram_parameter("bias", [128, 1], f32, isOutput=False)
    out = nc.declare_dram_parameter("out", [OUT_ROWS, OUT_COLS], f16, isOutput=True)

    # row groups: 8 x 126 + 1 x 16 = 1024
    groups = []
    m0 = 0
    while m0 < OUT_ROWS:
        m = min(M_TILE, OUT_ROWS - m0)
        groups.append((m0, m))
        m0 += m

    n_cols = OUT_COLS // N_TILE  # 4 column tiles per group

    with tile.TileContext(nc) as tc:
        with (
            tc.tile_pool(name="const", bufs=1) as cpool,
            tc.tile_pool(name="xin", bufs=4) as xpool,
            tc.tile_pool(name="psum", bufs=8, space="PSUM") as ppool,
            tc.tile_pool(name="oput", bufs=4) as opool,
        ):
            wt = cpool.tile([128, KW, M_TILE], f16)
            nc.sync.dma_start(wt[:], wb[:])
            bt = cpool.tile([128, 1], f32)
            nc.sync.dma_start(bt[:], bias[:])

            first_group = True
            for m0, m in groups:
                k = m + HALO
                xt = xpool.tile([128, IN_COLS], f16, tag="xin")
                if first_group:
                    # small leading load so the first matmul starts sooner
                    nc.sync.dma_start(xt[:k, :514], x[m0 : m0 + k, :514])
                    nc.sync.dma_start(xt[:k, 514:], x[m0 : m0 + k, 514:])
                    first_group = False
                else:
                    nc.sync.dma_start(xt[:k, :], x[m0 : m0 + k, :])
                ot = opool.tile([128, OUT_COLS], f16, tag="oput")
                for jj in range(n_cols):
                    c0 = jj * N_TILE
                    pt = ppool.tile([128, N_TILE], f32)
                    for b in range(KW):
                        nc.tensor.matmul(
                            pt[:m, :],
                            wt[:k, b, :m],
                            xt[:k, c0 + b : c0 + b + N_TILE],
                            start=(b == 0),
                            stop=(b == KW - 1),
                        )
                    # evict psum+bias to SBUF; alternate ACT/DVE
                    if jj % 2 == 0:
                        nc.scalar.activation(
                            ot[:m, c0 : c0 + N_TILE],
                            pt[:m, :],
                            mybir.ActivationFunctionType.Identity,
                            bias=bt[:m],
                            scale=1.0,
                        )
                    else:
                        nc.vector.tensor_scalar_add(
                            ot[:m, c0 : c0 + N_TILE], pt[:m, :], bt[:m]
                        )
                # stores go on the ACT HWDGE queue so their sem waits
                # don't head-of-line-block input loads on the SP queue
                nc.scalar.dma_start(out[m0 : m0 + m, :], ot[:m, :])

    nc.compile()
    return nc


def kernel(X: np.ndarray, weight: np.ndarray, bias: np.ndarray) -> np.ndarray:
    X = np.ascontiguousarray(X, dtype=np.float32)
    weight = np.asarray(weight, dtype=np.float32)
    bias = np.asarray(bias, dtype=np.float32)

    if "nc" not in _CACHE:
        _CACHE["nc"] = _build_program()
    nc = _CACHE["nc"]

    # host-side prep (tiny): padded fp16 image, banded weights, bias
    x_pad = np.zeros((H + HALO, W + HALO), dtype=np.float16)
    x_pad[:H, :W] = X.astype(np.float16)

    wb = np.zeros((128, KW, M_TILE), dtype=np.float16)
    m_idx = np.arange(M_TILE)
    for b in range(KW):
        for a in range(KH):
            wb[m_idx + a, b, m_idx] = weight[a, b].astype(np.float16)

    bias_bc = np.full((128, 1), bias[0], dtype=np.float32)

    in_maps = []
    for r in range(GRID_R):
        for c in range(GRID_C):
            in_maps.append(
                {
                    "x": np.ascontiguousarray(
                        x_pad[
                            r * OUT_ROWS : r * OUT_ROWS + IN_ROWS,
                            c * OUT_COLS : c * OUT_COLS + IN_COLS,
                        ]
                    ),
                    "wb": wb,
                    "bias": bias_bc,
                }
            )

    try:
        res = run_bass_kernel_spmd(nc, in_maps, core_ids=list(range(N_CORES)))
    except ModuleNotFoundError:
        # BASS_TRACE=1 requires the axon NTFF hook (antenv.axon_hooks),
        # which some containers lack -- fall back to an untraced run
        import os

        os.environ["BASS_NEVER_TRACE"] = "1"
        res = run_bass_kernel_spmd(nc, in_maps, core_ids=list(range(N_CORES)))
    _CACHE["last_results"] = res  # exec_time_ns when BASS_TRACE=1

    full = np.empty((H, W), dtype=np.float16)
    for r in range(GRID_R):
        for c in range(GRID_C):
            full[
                r * OUT_ROWS : (r + 1) * OUT_ROWS,
                c * OUT_COLS : (c + 1) * OUT_COLS,
            ] = res.results[r * GRID_C + c]["out"]
    return np.ascontiguousarray(
        full[: H - KH + 1, : W - KW + 1].astype(np.float32)
    )


# revision 27
# speedup vs baseline: 1.6317x; 1.0189x over previous
"""3x3 valid conv (cross-correlation) + bias on a 4096x4096 f32 image.

Sharding: a 4x2 grid over 8 NeuronCores -- 4 row-bands x 2 column-bands of
1024x2048 output each; the (kH-1) halo is provided host-side by
overlapping the per-core input slabs, so no device collective is needed.
The image is zero-padded to 4098x4098 so all cores run one uniform SPMD
program; pad regions are trimmed on gather. 4x2 beats 8x1 on the tensor
engine: the ragged <126-row tail group is paid once per 1024 rows instead
of once per 512 (108 matmuls/core vs 120).

Per-core compute (tensor engine): for each column shift b in {0,1,2}, a
banded [K=M+2, M] matrix B_b with B_b[m+a, m] = w[a, b] folds all three
row taps into the K-contraction:

    (B_b.T @ X_rows)[m, n] = sum_a w[a, b] * X[m+a, n]

Accumulating the three column-shifted views of the moving tensor into one
PSUM bank yields the full 3x3 conv in 3 matmuls per [126, 512] tile.
I/O is fp16 with f32 PSUM accumulation: the PE's fast fp32 path (fp32r)
already rounds operands to ~12 mantissa bits, so fp16 inputs cost almost
nothing extra, and the fp16 store (upcast to f32 on host) trades ~2x
absmax error (5.8e-4 scale-relative, resid_var 7e-8 -- still ~1e3 inside
the 1e-4 resid_var gate) for half the store traffic. Bias is fused into
the PSUM->SBUF eviction, which alternates between scalar and vector
engines.

DMA layout (4.3 MB in + 4.2 MB out per core at 360 GB/s): input loads
ride the SP HWDGE ring, output stores the ACT HWDGE ring, so store
sem-waits never head-of-line-block loads; 4-deep pools let loads run
groups ahead. Cost-model timeline: ~30 us per core.
"""

import sys

if "/opt/trn_rl_repo" not in sys.path:
    sys.path.insert(0, "/opt/trn_rl_repo")

import numpy as np

import concourse.bacc as bacc
import concourse.mybir as mybir
from concourse import tile
from concourse.bass_utils import run_bass_kernel_spmd

N_CORES = 8
GRID_R, GRID_C = 4, 2  # 4 row-bands x 2 col-bands
H, W = 4096, 4096
KH, KW = 3, 3
HALO = 2  # KH - 1
OUT_ROWS = H // GRID_R  # 1024 output rows per core (padded)
OUT_COLS = W // GRID_C  # 2048 output cols per core (padded)
IN_ROWS = OUT_ROWS + HALO  # 1026
IN_COLS = OUT_COLS + HALO  # 2050
M_TILE = 126  # output rows per matmul (K = M + 2 <= 128)
N_TILE = 512  # matmul free dim = one PSUM bank of f32

_CACHE = {}


def _build_program():
    f32 = mybir.dt.float32
    f16 = mybir.dt.float16

    nc = bacc.Bacc(
        "TRN2", target_bir_lowering=False, debug=False, num_devices=N_CORES
    )
    x = nc.declare_dram_parameter("x", [IN_ROWS, IN_COLS], f16, isOutput=False)
    wb = nc.declare_dram_parameter(
        "wb", [128, KW * M_TILE + 2], f16, isOutput=False
    )
    out = nc.declare_dram_parameter("out", [OUT_ROWS, OUT_COLS], f16, isOutput=True)

    # row groups: 8 x 126 + 1 x 16 = 1024
    groups = []
    m0 = 0
    while m0 < OUT_ROWS:
        m = min(M_TILE, OUT_ROWS - m0)
        groups.append((m0, m))
        m0 += m

    n_cols = OUT_COLS // N_TILE  # 4 column tiles per group

    with tile.TileContext(nc) as tc:
        with (
            tc.tile_pool(name="const", bufs=1) as cpool,
            tc.tile_pool(name="xin", bufs=4) as xpool,
            tc.tile_pool(name="psum", bufs=8, space="PSUM") as ppool,
            tc.tile_pool(name="oput", bufs=4) as opool,
        ):
            wt = cpool.tile([128, KW * M_TILE + 2], f16)
            nc.sync.dma_start(wt[:], wb[:])
            # bias rides in wt's last 2 fp16 slots as raw f32 bits
            def bt(mm):
                return wt[:mm, KW * M_TILE : KW * M_TILE + 2].bitcast(f32)

            first_group = True
            for m0, m in groups:
                k = m + HALO
                xt = xpool.tile([128, IN_COLS], f16, tag="xin")
                if first_group:
                    # small leading load so the first matmul starts sooner
                    nc.sync.dma_start(xt[:k, :514], x[m0 : m0 + k, :514])
                    nc.sync.dma_start(xt[:k, 514:], x[m0 : m0 + k, 514:])
                    first_group = False
                else:
                    nc.sync.dma_start(xt[:k, :], x[m0 : m0 + k, :])
                ot = opool.tile([128, OUT_COLS], f16, tag="oput")
                for jj in range(n_cols):
                    c0 = jj * N_TILE
                    pt = ppool.tile([128, N_TILE], f32)
                    for b in range(KW):
                        nc.tensor.matmul(
                            pt[:m, :],
                            wt[:k, b * M_TILE : b * M_TILE + m],
                            xt[:k, c0 + b : c0 + b + N_TILE],
                            start=(b == 0),
                            stop=(b == KW - 1),
                        )
                    # evict psum+bias to SBUF; alternate ACT/DVE
                    if jj % 2 == 0:
                        nc.scalar.activation(
                            ot[:m, c0 : c0 + N_TILE],
                            pt[:m, :],
                            mybir.ActivationFunctionType.Identity,
                            bias=bt(m),
                            scale=1.0,
                        )
                    else:
                        nc.vector.tensor_scalar_add(
                            ot[:m, c0 : c0 + N_TILE], pt[:m, :], bt(m)
                        )
                # stores go on the ACT HWDGE queue so their sem waits
                # don't head-of-line-block input loads on the SP queue
                nc.scalar.dma_start(out[m0 : m0 + m, :], ot[:m, :])

    nc.compile()
    return nc


def kernel(X: np.ndarray, weight: np.ndarray, bias: np.ndarray) -> np.ndarray:
    X = np.ascontiguousarray(X, dtype=np.float32)
    weight = np.asarray(weight, dtype=np.float32)
    bias = np.asarray(bias, dtype=np.float32)

    if "nc" not in _CACHE:
        _CACHE["nc"] = _build_program()
    nc = _CACHE["nc"]

    # host-side prep (tiny): padded fp16 image, banded weights, bias
    x_pad = np.zeros((H + HALO, W + HALO), dtype=np.float16)
    x_pad[:H, :W] = X.astype(np.float16)

    wb3 = np.zeros((128, KW, M_TILE), dtype=np.float16)
    m_idx = np.arange(M_TILE)
    for b in range(KW):
        for a in range(KH):
            wb3[m_idx + a, b, m_idx] = weight[a, b].astype(np.float16)
    wb = np.zeros((128, KW * M_TILE + 2), dtype=np.float16)
    wb[:, : KW * M_TILE] = wb3.reshape(128, -1)
    wb[:, KW * M_TILE :] = (
        np.full((128, 1), bias[0], dtype=np.float32).view(np.float16)
    )

    in_maps = []
    for r in range(GRID_R):
        for c in range(GRID_C):
            in_maps.append(
                {
                    "x": np.ascontiguousarray(
                        x_pad[
                            r * OUT_ROWS : r * OUT_ROWS + IN_ROWS,
                            c * OUT_COLS : c * OUT_COLS + IN_COLS,
                        ]
                    ),
                    "wb": wb,
                }
            )

    try:
        res = run_bass_kernel_spmd(nc, in_maps, core_ids=list(range(N_CORES)))
    except ModuleNotFoundError:
        # BASS_TRACE=1 requires the axon NTFF hook (antenv.axon_hooks),
        # which some containers lack -- fall back to an untraced run
        import os

        os.environ["BASS_NEVER_TRACE"] = "1"
        res = run_bass_kernel_spmd(nc, in_maps, core_ids=list(range(N_CORES)))
    _CACHE["last_results"] = res  # exec_time_ns when BASS_TRACE=1

    full = np.empty((H, W), dtype=np.float16)
    for r in range(GRID_R):
        for c in range(GRID_C):
            full[
                r * OUT_ROWS : (r + 1) * OUT_ROWS,
                c * OUT_COLS : (c + 1) * OUT_COLS,
            ] = res.results[r * GRID_C + c]["out"]
    return np.ascontiguousarray(
        full[: H - KH + 1, : W - KW + 1].astype(np.float32)
    )
